# revision 6
# baseline (speedup 1.0000x reference)
"""Cross-modal attention block on 8 Trainium2 NeuronCores.

Sharding: core = 2*b + g  ->  batch b (4-way data parallel) x head-group g
(2-way tensor parallel over 16 heads -> 8 heads/core).  Each core:
  rownorm(x[b]) -> PE transpose -> q projection (ternary weights, gamma/beta
  folded) ; kT/v projections from pre-transposed context ; per-head
  scoresT = k~^T q~ (K=64 matmuls), exp on ScalarE, unnormalized attn-out
  with an appended ones-row producing softmax denominators in the same
  matmul ; normalize ; out-proj partial.  Host sums the two partials per
  batch + residual + folded biases.

All matmuls run in float32r (fp32 data, ~13-bit-mantissa PE path, 1 cyc/row).
"""

import os

import numpy as np

import concourse.bass as bass
import concourse.mybir as mybir
import concourse.tile as tile
from concourse import bacc
from concourse.bass_utils import run_bass_kernel_spmd
from concourse.masks import make_identity

FP = mybir.dt.float32
FPR = mybir.dt.float32r

B, T, TC, C = 4, 1024, 2048, 1024
H, HD = 16, 64
HL = 8           # heads per core
CL = HL * HD     # 512 local channels
SCALE = HD ** -0.5
LN_EPS = 1e-5
Q_EPS = 1e-5
P = 128
NCORES = 8

last_exec_time_ns = None


def _build_nc():
    nc = bacc.Bacc(None, target_bir_lowering=False, debug=False)

    x_d = nc.dram_tensor("x", [T // P, P, C], FP, kind="ExternalInput")
    ctxT_d = nc.dram_tensor("ctxT", [P, C // P, TC], FPR, kind="ExternalInput")
    wqT_d = nc.dram_tensor("wqT", [P, C // P, CL], FPR, kind="ExternalInput")
    wkT_d = nc.dram_tensor("wkT", [P, C // P, CL], FPR, kind="ExternalInput")
    wvT_d = nc.dram_tensor("wvT", [P, C // P, CL], FPR, kind="ExternalInput")
    woT_d = nc.dram_tensor("woT", [P, CL // P, C], FPR, kind="ExternalInput")
    cb_d = nc.dram_tensor("cb", [P, 9], FP, kind="ExternalInput")
    part_d = nc.dram_tensor("partial", [C // P, P, T], FP, kind="ExternalOutput")

    NT = T // P            # 8 query-row tiles
    NKC = C // P           # 8 contraction chunks over C
    NJ = TC // P           # 16 context chunks
    NM = CL // P           # 4 local d-chunks
    NH = T // 512          # 2 query halves

    with tile.TileContext(nc) as tc:
        with (
            tc.tile_pool(name="const", bufs=1) as cpool,
            tc.tile_pool(name="acts", bufs=1) as apool,
            tc.tile_pool(name="psmm", bufs=3, space="PSUM") as psmm,
        ):
            ident_f = cpool.tile([P, P], FP)
            make_identity(nc, ident_f[:])
            ident = cpool.tile([P, P], FPR)
            nc.vector.tensor_copy(ident[:], ident_f[:])
            ones_f = cpool.tile([P, P], FP)
            nc.vector.memset(ones_f[:], 1.0)
            ones_r = cpool.tile([P, P], FPR)
            nc.vector.tensor_copy(ones_r[:], ones_f[:])
            cb = cpool.tile([P, 9], FP)
            nc.sync.dma_start(cb[:], cb_d[:])
            eps = cpool.tile([P, 1], FP)
            nc.vector.memset(eps[:], LN_EPS)

            qT = apool.tile([P, NM, T], FPR, tag="qT")
            kT = apool.tile([P, NM, TC], FPR, tag="kT")
            vv = apool.tile([P, NJ, HL * (HD + 1)], FPR, tag="vv")

            # ones column of v' (denominator rows), written once
            nc.vector.tensor_copy(
                vv[:].rearrange("p j (h c) -> p (j h) c", c=HD + 1)[:, :, HD : HD + 1],
                ones_r[:, 0 : NJ * HL][:, :, None],
            )

            # ---- phase A1: rownorm + transpose + q projection ----
            with (
                tc.tile_pool(name="xrn", bufs=2) as xpool,
                tc.tile_pool(name="xst", bufs=3) as spool,
                tc.tile_pool(name="rnt", bufs=1) as rpool,
                tc.tile_pool(name="wqp", bufs=1) as wqpool,
                tc.tile_pool(name="pstr", bufs=2, space="PSUM") as pstr,
            ):
                wq = wqpool.tile([P, NKC, CL], FPR, tag="wq")
                nc.sync.dma_start(wq[:], wqT_d[:])
                rnT = rpool.tile([P, NKC, T], FPR, tag="rnT")
                for t in range(NT):
                    xt = xpool.tile([P, C], FP, tag="xt")
                    nc.sync.dma_start(xt[:], x_d[t])
                    nmu = spool.tile([P, 1], FP, tag="nmu")
                    nc.vector.reduce_sum(nmu[:], xt[:], axis=mybir.AxisListType.X)
                    nc.scalar.mul(nmu[:], nmu[:], -1.0 / C)
                    sq = xpool.tile([P, C], FP, tag="sq")
                    ex2 = spool.tile([P, 1], FP, tag="ex2")
                    nc.scalar.activation(
                        sq[:], xt[:], mybir.ActivationFunctionType.Square,
                        accum_out=ex2[:],
                    )
                    var = spool.tile([P, 1], FP, tag="var")
                    nc.scalar.mul(ex2[:], ex2[:], 1.0 / C)
                    mu2 = spool.tile([P, 1], FP, tag="mu2")
                    nc.vector.tensor_mul(mu2[:], nmu[:], nmu[:])
                    nc.vector.tensor_sub(var[:], ex2[:], mu2[:])
                    std = spool.tile([P, 1], FP, tag="std")
                    nc.scalar.activation(
                        std[:], var[:], mybir.ActivationFunctionType.Sqrt,
                        bias=eps[:],
                    )
                    inv = spool.tile([P, 1], FP, tag="inv")
                    nc.vector.reciprocal(inv[:], std[:])
                    rn = xpool.tile([P, C], FPR, tag="rn")
                    nc.vector.scalar_tensor_tensor(
                        out=rn[:], in0=xt[:], scalar=nmu[:],
                        in1=inv[:].to_broadcast((P, C)),
                        op0=mybir.AluOpType.add, op1=mybir.AluOpType.mult,
                    )
                    for c in range(NKC):
                        pt = pstr.tile([P, P], FP, tag="ptr")
                        nc.tensor.transpose(
                            pt[:].bitcast(FPR), rn[:, c * P : (c + 1) * P],
                            ident[:],
                        )
                        nc.scalar.copy(rnT[:, c, t * P : (t + 1) * P], pt[:])

                # ---- q projection: qT[m] += wq[k,m]^T @ rnT[k] ----
                for m in range(NM):
                    for n in range(2):
                        ps = psmm.tile([P, 512], FP, tag="mm")
                        for k in range(NKC):
                            nc.tensor.matmul(
                                ps[:],
                                wq[:, k, m * P : (m + 1) * P],
                                rnT[:, k, n * 512 : (n + 1) * 512],
                                start=(k == 0), stop=(k == NKC - 1),
                            )
                        nc.vector.tensor_scalar(
                            out=qT[:, m, n * 512 : (n + 1) * 512], in0=ps[:],
                            scalar1=cb[:, m : m + 1], scalar2=cb[:, 8:9],
                            op0=mybir.AluOpType.add, op1=mybir.AluOpType.mult,
                        )

            # ---- phase A2: k/v projections, context streamed in halves ----
            with (
                tc.tile_pool(name="ctx", bufs=1) as ctxpool,
                tc.tile_pool(name="wkv", bufs=1) as wpool,
            ):
                wk = wpool.tile([P, NKC, CL], FPR, tag="wk")
                wv = wpool.tile([P, NKC, CL], FPR, tag="wv")
                nc.sync.dma_start(wk[:], wkT_d[:])
                nc.sync.dma_start(wv[:], wvT_d[:])
                for ch in range(2):
                    ctxT = ctxpool.tile([P, NKC, TC // 2], FPR, tag="ctxT")
                    for k in range(NKC):
                        nc.sync.dma_start(
                            ctxT[:, k, :], ctxT_d[:, k, ch * (TC // 2) : (ch + 1) * (TC // 2)],
                        )
                    # k projection for this context half
                    for m in range(NM):
                        for n2 in range(2):
                            n = 2 * ch + n2
                            ps = psmm.tile([P, 512], FP, tag="mm")
                            for k in range(NKC):
                                nc.tensor.matmul(
                                    ps[:],
                                    wk[:, k, m * P : (m + 1) * P],
                                    ctxT[:, k, n2 * 512 : (n2 + 1) * 512],
                                    start=(k == 0), stop=(k == NKC - 1),
                                )
                            nc.vector.tensor_scalar_add(
                                kT[:, m, n * 512 : (n + 1) * 512], ps[:],
                                cb[:, 4 + m : 5 + m],
                            )
                    # v projection for this context half
                    for jj in range(NJ // 2):
                        j = ch * (NJ // 2) + jj
                        ps = psmm.tile([P, 512], FP, tag="mm")
                        for k in range(NKC):
                            nc.tensor.matmul(
                                ps[:],
                                ctxT[:, k, jj * P : (jj + 1) * P],
                                wv[:, k, :],
                                start=(k == 0), stop=(k == NKC - 1),
                            )
                        nc.vector.tensor_copy(
                            vv[:, j, :].rearrange("p (h c) -> p h c", c=HD + 1)[:, :, 0:HD],
                            ps[:].rearrange("p (h c) -> p h c", c=HD),
                        )

            # ---- attention + out-proj ----
            with (
                tc.tile_pool(name="wo", bufs=1) as wopool,
                tc.tile_pool(name="att", bufs=1) as attpool,
                tc.tile_pool(name="exp", bufs=6) as epool,
                tc.tile_pool(name="nrm", bufs=3) as npool,
                tc.tile_pool(name="oev", bufs=3) as opool,
                tc.tile_pool(name="psat", bufs=2, space="PSUM") as psat,
                tc.tile_pool(name="psbc", bufs=1, space="PSUM") as psbc,
            ):
                wo = wopool.tile([P, NM, C], FPR, tag="wo")
                nc.sync.dma_start(wo[:], woT_d[:])

                for half in range(NH):
                    hs = slice(half * 512, (half + 1) * 512)
                    attnT = attpool.tile([P, NM, 512], FPR, tag="attnT")
                    for h in range(HL):
                        prow = 64 * (h % 2)
                        mh = h // 2
                        ph = psat.tile([HD + 1, 512], FP, tag="ph")
                        for j in range(NJ):
                            pscr = psmm.tile([P, 512], FP, tag="mm")
                            nc.tensor.matmul(
                                pscr[:],
                                kT[prow : prow + HD, mh, j * P : (j + 1) * P],
                                qT[prow : prow + HD, mh, hs],
                                start=True, stop=True,
                            )
                            et = epool.tile([P, 512], FPR, tag="et")
                            nc.scalar.activation(
                                et[:], pscr[:], mybir.ActivationFunctionType.Exp,
                            )
                            nc.tensor.matmul(
                                ph[:],
                                vv[:, j, h * (HD + 1) : (h + 1) * (HD + 1)],
                                et[:],
                                start=(j == 0), stop=(j == NJ - 1),
                            )
                        sr = npool.tile([1, 512], FPR, tag="sr")
                        nc.vector.tensor_copy(sr[:], ph[HD : HD + 1, :])
                        pb = psbc.tile([HD, 512], FP, tag="pb")
                        nc.tensor.matmul(
                            pb[:], ones_r[0:1, 0:HD], sr[:], start=True, stop=True,
                        )
                        rec = npool.tile([HD, 512], FP, tag="rec")
                        nc.vector.reciprocal(rec[:], pb[:])
                        nc.vector.tensor_mul(
                            attnT[prow : prow + HD, mh, :], ph[0:HD, :], rec[:],
                        )
                    # out-proj partial for this half
                    for m in range(C // P):
                        po = psmm.tile([P, 512], FP, tag="mm")
                        for k2 in range(NM):
                            nc.tensor.matmul(
                                po[:],
                                wo[:, k2, m * P : (m + 1) * P],
                                attnT[:, k2, :],
                                start=(k2 == 0), stop=(k2 == NM - 1),
                            )
                        ot = opool.tile([P, 512], FP, tag="ot")
                        nc.vector.tensor_copy(ot[:], po[:])
                        nc.sync.dma_start(part_d[m, :, hs], ot[:])

    nc.finalize()
    return nc


_NC_CACHE = {}


def _get_nc():
    if "nc" not in _NC_CACHE:
        _NC_CACHE["nc"] = _build_nc()
    return _NC_CACHE["nc"]


def _quant(w):
    g = np.float32(np.mean(np.abs(w), dtype=np.float64))
    t = np.clip(np.rint(w / (g + np.float32(Q_EPS))), -1.0, 1.0).astype(np.float32)
    return t, g


def _pack_kp(a):
    # [K, M] -> [P, K//P, M] (partition-major chunks)
    k, m = a.shape
    return np.ascontiguousarray(a.reshape(k // P, P, m).transpose(1, 0, 2))


def kernel(**inputs):
    global last_exec_time_ns
    x = np.asarray(inputs["x"], dtype=np.float32)
    ctx = np.asarray(inputs["context"], dtype=np.float32)
    Wq = np.asarray(inputs["Wq"], dtype=np.float32)
    Wk = np.asarray(inputs["Wk"], dtype=np.float32)
    Wv = np.asarray(inputs["Wv"], dtype=np.float32)
    Wo = np.asarray(inputs["Wo"], dtype=np.float32)
    bq = np.asarray(inputs["bq"], dtype=np.float32)
    bk = np.asarray(inputs["bk"], dtype=np.float32)
    bv = np.asarray(inputs["bv"], dtype=np.float32)
    bo = np.asarray(inputs["bo"], dtype=np.float32)
    g_ln = np.asarray(inputs["ln_gamma"], dtype=np.float32)
    b_ln = np.asarray(inputs["ln_beta"], dtype=np.float32)

    Tq, gq = _quant(Wq)
    Tk, gk = _quant(Wk)
    Tv, gv = _quant(Wv)
    To, go = _quant(Wo)

    qb_full = (bq + b_ln @ (gq * Tq).T) / gq          # [C]
    scale = np.float32(gq * gk * SCALE)
    host_bias = bo + bv @ (go * To).T                 # [C]

    in_maps = []
    for core in range(NCORES):
        b = core // 2
        g = core % 2
        rows = slice(CL * g, CL * (g + 1))
        wqT = _pack_kp((Tq[rows] * g_ln[None, :]).T)  # [P, 8, 512]
        wkT = _pack_kp(Tk[rows].T)
        wvT = _pack_kp(Tv[rows].T)
        woT = _pack_kp((To[:, rows] * (go * gv)).T)   # [P, 4, 1024]
        cbm = np.zeros((P, 9), dtype=np.float32)
        cbm[:, 0:4] = qb_full[rows].reshape(4, P).T
        cbm[:, 4:8] = (bk[rows] / gk).reshape(4, P).T
        cbm[:, 8] = scale
        in_maps.append({
            "x": np.ascontiguousarray(x[b].reshape(T // P, P, C)),
            "ctxT": _pack_kp(np.ascontiguousarray(ctx[b].T)),
            "wqT": wqT, "wkT": wkT, "wvT": wvT, "woT": woT,
            "cb": cbm,
        })

    nc = _get_nc()
    trace = os.environ.get("KERNEL_TRACE", "0") == "1"
    res = run_bass_kernel_spmd(nc, in_maps, list(range(NCORES)), trace=trace)
    last_exec_time_ns = res.exec_time_ns

    out = np.empty((B, T, C), dtype=np.float32)
    for b in range(B):
        p0 = res.results[2 * b]["partial"].reshape(C, T)
        p1 = res.results[2 * b + 1]["partial"].reshape(C, T)
        out[b] = x[b] + p0.T + p1.T + host_bias[None, :]
    return out
